# revision 9
# baseline (speedup 1.0000x reference)
"""CosineDistanceLoss kernel for Trainium2 (8 NeuronCores, Bass/Tile).

reference: mean_n(1 - sum_d feats[d,n] * warped_feats[d,n])
         = 1 - (1/N) * sum_{d,n} feats[d,n] * warped_feats[d,n]

The loss is a single global sum of the elementwise product, so any disjoint
partition of elements across cores is valid. We shard along D (rows): core c
gets rows [64c, 64c+64) of both tensors - contiguous views, zero host copy.
Each 64 x 65536 shard is a flat 4.19M-element buffer viewed as [128, 32768]
so SBUF tiles use all 128 partitions. The kernel streams [128, FCHUNK] chunks
of both tensors and runs one fused DVE tensor_tensor_reduce (elementwise mult
+ free-axis add-reduce, product never materialized - broadcast dummy out) per
chunk, accumulating per-partition partial sums. Host combines in f64.
"""

import numpy as np

import concourse.bacc as bacc
import concourse.mybir as mybir
from concourse.tile import TileContext
from concourse.bass_utils import run_bass_kernel_spmd

D, N = 512, 65536
NCORES = 8
DSHARD = D // NCORES            # 64 rows per core
P = 128                         # SBUF partitions
M = DSHARD * N // P             # 32768 free elements per partition
FCHUNK = 4096                   # free-dim chunk per DMA/compute step
NCHUNK = M // FCHUNK            # 8 chunks
DMA_BUFS = 3                    # per-tensor double/triple buffering

_CACHE = {}


def _build_bass():
    # Bacc (not plain Bass): its compile() pipeline runs
    # generate_event_semaphores (TRN2 allows at most 1 sem wait per compute
    # instruction; extra waits must be split into standalone EventSemaphore
    # instructions) and codegen_inst_isa_subclasses.
    nc = bacc.Bacc(None)
    f_in = nc.declare_dram_parameter("feats", [P, M], mybir.dt.float32, isOutput=False)
    w_in = nc.declare_dram_parameter("warped", [P, M], mybir.dt.float32, isOutput=False)
    out = nc.declare_dram_parameter(
        "partial", [P, NCHUNK], mybir.dt.float32, isOutput=True
    )

    with TileContext(nc) as tc:
        with (
            tc.tile_pool(name="accp", bufs=1) as accp,
            tc.tile_pool(name="fp", bufs=DMA_BUFS) as fp,
            tc.tile_pool(name="wp", bufs=DMA_BUFS) as wp,
            tc.tile_pool(name="dp", bufs=NCHUNK) as dp,
        ):
            acc = accp.tile([P, NCHUNK], mybir.dt.float32)
            for j in range(NCHUNK):
                ft = fp.tile([P, FCHUNK], mybir.dt.float32)
                wt = wp.tile([P, FCHUNK], mybir.dt.float32)
                nc.sync.dma_start(ft[:, :], f_in[:, j * FCHUNK : (j + 1) * FCHUNK])
                nc.sync.dma_start(wt[:, :], w_in[:, j * FCHUNK : (j + 1) * FCHUNK])
                dummy = dp.tile([P, 1], mybir.dt.float32)
                # out = (ft * 1.0) * wt (discarded via stride-0 broadcast),
                # accum_out = per-partition sum of the product. Native BIR
                # TensorScalarPtr - unlike InstTensorTensorReduce it can
                # carry both input-DMA sem waits.
                nc.vector.scalar_tensor_tensor(
                    dummy.broadcast_to((P, FCHUNK)),
                    ft[:, :],
                    1.0,
                    wt[:, :],
                    op0=mybir.AluOpType.mult,
                    op1=mybir.AluOpType.mult,
                    accum_out=acc[:, j : j + 1],
                )
            nc.sync.dma_start(out[:, :], acc[:, :])

    # Bacc defers register allocation and wait-splitting to compile(), which
    # finalize() runs; the spmd runner serializes nc.m as-is.
    nc.finalize()
    return nc


def _get_nc():
    if "nc" not in _CACHE:
        _CACHE["nc"] = _build_bass()
    return _CACHE["nc"]


def _run(feats, warped_feats, **spmd_kwargs):
    feats = np.ascontiguousarray(np.asarray(feats), dtype=np.float32)
    warped = np.ascontiguousarray(np.asarray(warped_feats), dtype=np.float32)
    assert feats.shape == (D, N) and warped.shape == (D, N)

    in_maps = [
        {
            "feats": feats[c * DSHARD : (c + 1) * DSHARD].reshape(P, M),
            "warped": warped[c * DSHARD : (c + 1) * DSHARD].reshape(P, M),
        }
        for c in range(NCORES)
    ]
    return run_bass_kernel_spmd(
        _get_nc(), in_maps, core_ids=list(range(NCORES)), **spmd_kwargs
    )


def kernel(feats, warped_feats):
    res = _run(feats, warped_feats)
    total = 0.0
    for r in res.results:
        total += float(r["partial"].astype(np.float64).sum())
    return np.array(1.0 - total / N, dtype=np.float32)
